# revision 2
# baseline (speedup 1.0000x reference)
"""DeepSeek-MoE SwiGLU expert layer on 8 TRN2 NeuronCores (expert parallelism).

Strategy (hardcoded for T=4096, D=1024, DFF=1408, E=8, K=2, 8 cores):
  - Expert parallelism: core e holds expert e's (Wg, Wu, Wd).
  - Dispatch happens at input-sharding time on the host: for each expert,
    gather the tokens routed to it (deduped via the combine matrix), pad to
    capacity C, and ship X^T [D, C] to that core.  Shipping X transposed
    makes every matmul operand on-device natural-layout (contraction dim =
    partition dim), so the kernel needs zero transposes.
  - bf16 everywhere on device (measured sustained PE rate ~0.72 ns/row vs
    0.83 for fp32r; absmax rel err ~4e-3, well under the 2e-2 gate).
    Weights are pre-tiled host-side into [ftile, p, k*m] layout so each
    weight DMA is fully contiguous per partition line; weight/x/y DMAs are
    round-robined across the two HWDGE queues (sync + scalar engines).
  - Per core:  HT = silu(Wg^T @ XT) * (Wu^T @ XT)   [DFF, C]
               YT = Wd^T @ HT                        [D, C]
    fp32 PSUM accumulation.
  - Combine on host: out[idx_e] += (YT[:, :cnt]).T * combine_weight.
"""

import numpy as np
import ml_dtypes
from contextlib import ExitStack

import concourse.bass as bass
import concourse.tile as tile
from concourse import bacc, mybir
from concourse import bass_utils

T, D, DFF, E = 4096, 1024, 1408, 8
N_CORES = 8
P = 128
CT = 512  # matmul moving-operand width (one PSUM bank of fp32)
KD = D // P    # 8 k-tiles over D
KF = DFF // P  # 11 k-tiles over DFF

BF16 = mybir.dt.bfloat16
NP_BF16 = ml_dtypes.bfloat16

_cache = {}


def _c_tiles(C):
    tiles = []
    off = 0
    while off < C:
        w = min(CT, C - off)
        tiles.append((off, w))
        off += w
    return tiles


def _emit_body(nc, pools, aps, C):
    f32 = mybir.dt.float32
    ctiles = _c_tiles(C)
    xp, hp, wp, pp, sp, op = pools
    xt, wg, wu, wd, yt = aps
    Silu = mybir.ActivationFunctionType.Silu

    qs = (nc.sync, nc.scalar)
    qn = [0]

    def q():
        e = qs[qn[0] & 1]
        qn[0] += 1
        return e

    def load_w1(f):
        wg_sl = wp.tile([P, KD, P], BF16, tag="wg", name=f"wg_sl{f}")
        q().dma_start(out=wg_sl[:],
                      in_=wg[f].rearrange("p (k m) -> p k m", k=KD))
        wu_sl = wp.tile([P, KD, P], BF16, tag="wu", name=f"wu_sl{f}")
        q().dma_start(out=wu_sl[:],
                      in_=wu[f].rearrange("p (k m) -> p k m", k=KD))
        return wg_sl, wu_sl

    # Issue the first f-tile's weight DMAs before the (larger) XT chunk DMAs
    # so the first matmul group isn't queued behind all of XT.
    w1_0 = load_w1(0)

    xt3 = xt.rearrange("(k p) c -> p k c", p=P)
    x_sb = {}
    for i, (c0, cw) in enumerate(ctiles):
        for k in range(KD):
            x_sb[i, k] = xp.tile([P, cw], BF16, tag=f"x{i}k{k}",
                                 name=f"x_sb{i}_{k}")
            q().dma_start(out=x_sb[i, k][:], in_=xt3[:, k, c0:c0 + cw])

    h_sb = {}
    for i, (c0, cw) in enumerate(ctiles):
        h_sb[i] = hp.tile([P, KF, cw], BF16, tag=f"h{i}", name=f"h_sb{i}")

    # stage 1: HT[f, c] = silu(Wg^T XT) * (Wu^T XT), transposed space
    for f in range(KF):
        wg_sl, wu_sl = w1_0 if f == 0 else load_w1(f)
        for i, (c0, cw) in enumerate(ctiles):
            ps_g = pp.tile([P, CT], f32, tag="psg")
            ps_u = pp.tile([P, CT], f32, tag="psu")
            for k in range(KD):
                nc.tensor.matmul(ps_g[:, :cw], lhsT=wg_sl[:, k, :],
                                 rhs=x_sb[i, k][:],
                                 start=(k == 0), stop=(k == KD - 1))
            for k in range(KD):
                nc.tensor.matmul(ps_u[:, :cw], lhsT=wu_sl[:, k, :],
                                 rhs=x_sb[i, k][:],
                                 start=(k == 0), stop=(k == KD - 1))
            sg = sp.tile([P, CT], f32)
            nc.scalar.activation(sg[:, :cw], ps_g[:, :cw], Silu)
            nc.vector.tensor_mul(h_sb[i][:, f, :], sg[:, :cw], ps_u[:, :cw])

    # stage 2: YT[dout, c] = Wd^T @ HT
    for do in range(KD):
        wd_sl = wp.tile([P, KF, P], BF16, tag="wd")
        q().dma_start(out=wd_sl[:],
                      in_=wd[do].rearrange("p (k m) -> p k m", k=KF))
        for i, (c0, cw) in enumerate(ctiles):
            ps_y = pp.tile([P, CT], f32, tag="psy")
            for k in range(KF):
                nc.tensor.matmul(ps_y[:, :cw], lhsT=wd_sl[:, k, :],
                                 rhs=h_sb[i][:, k, :],
                                 start=(k == 0), stop=(k == KF - 1))
            y_sb = op.tile([P, CT], f32)
            nc.vector.tensor_copy(y_sb[:, :cw], ps_y[:, :cw])
            q().dma_start(out=yt[do * P:(do + 1) * P, c0:c0 + cw],
                          in_=y_sb[:, :cw])


def _declare(nc, C):
    f32 = mybir.dt.float32
    xt = nc.dram_tensor("xt", [D, C], BF16, kind="ExternalInput").ap()
    wg = nc.dram_tensor("wg", [KF, P, KD * P], BF16,
                        kind="ExternalInput").ap()
    wu = nc.dram_tensor("wu", [KF, P, KD * P], BF16,
                        kind="ExternalInput").ap()
    wd = nc.dram_tensor("wd", [KD, P, KF * P], BF16,
                        kind="ExternalInput").ap()
    yt = nc.dram_tensor("yt", [D, C], f32, kind="ExternalOutput").ap()
    return (xt, wg, wu, wd, yt)


def _pools(tc, ctx):
    xp = ctx.enter_context(tc.tile_pool(name="xt_p", bufs=1))
    hp = ctx.enter_context(tc.tile_pool(name="ht_p", bufs=1))
    wp = ctx.enter_context(tc.tile_pool(name="w_p", bufs=4))
    pp = ctx.enter_context(tc.tile_pool(name="ps_p", bufs=2, space="PSUM"))
    sp = ctx.enter_context(tc.tile_pool(name="sg_p", bufs=4))
    op = ctx.enter_context(tc.tile_pool(name="y_p", bufs=4))
    return (xp, hp, wp, pp, sp, op)


def _build(C):
    key = ("plain", C)
    if key in _cache:
        return _cache[key]
    nc = bacc.Bacc("TRN2", target_bir_lowering=False, debug=False,
                   num_devices=N_CORES)
    aps = _declare(nc, C)
    with tile.TileContext(nc) as tc, ExitStack() as ctx:
        pools = _pools(tc, ctx)
        _emit_body(nc, pools, aps, C)
    nc.compile()
    _cache[key] = nc
    return nc


def _build_loop(C):
    """Benchmark variant: repeat the body niter times (runtime input)."""
    key = ("loop", C)
    if key in _cache:
        return _cache[key]
    nc = bacc.Bacc("TRN2", target_bir_lowering=False, debug=False,
                   num_devices=N_CORES)
    aps = _declare(nc, C)
    n_ap = nc.dram_tensor("niter", [1, 1], mybir.dt.uint32,
                          kind="ExternalInput").ap()
    with tile.TileContext(nc) as tc, ExitStack() as ctx:
        cpool = ctx.enter_context(tc.tile_pool(name="c_p", bufs=1))
        pools = _pools(tc, ctx)
        n_sb = cpool.tile([1, 1], mybir.dt.uint32)
        nc.sync.dma_start(out=n_sb[:], in_=n_ap[:])
        with tc.tile_critical():
            tmp = nc.alloc_registers("niter_regs")
            nc.regs_load(tmp, n_sb[0:1, 0:1])
            n_val = nc.snap(tmp, donate=True, min_val=0, max_val=1 << 20)
        with tc.For_i(0, n_val, 1, hint_engines=(mybir.EngineType.PE,)):
            _emit_body(nc, pools, aps, C)
    nc.compile()
    _cache[key] = nc
    return nc


def _dispatch(x, topk_weights, topk_indices, num_experts):
    """Host-side routing: combine matrix + per-expert token index lists."""
    T_, _ = x.shape
    E_ = int(num_experts)
    ti = np.asarray(topk_indices).astype(np.int64)
    tw = np.asarray(topk_weights).astype(np.float32)
    combine = np.zeros((T_, E_), np.float32)
    np.add.at(combine, (np.arange(T_)[:, None], ti), tw)
    idxs = [np.nonzero(combine[:, e])[0] for e in range(E_)]
    return combine, idxs


def _capacity(idxs):
    maxc = max((len(i) for i in idxs), default=0)
    return max(CT, ((maxc + P - 1) // P) * P)


def _in_maps(x, Wg, Wu, Wd, idxs, C):
    maps = []
    D_ = x.shape[1]
    for e in range(len(idxs)):
        xt_e = np.zeros((D_, C), NP_BF16)
        n = len(idxs[e])
        if n:
            xt_e[:, :n] = x[idxs[e]].T.astype(NP_BF16)
        wg_t = (Wg[e].reshape(KD, P, KF, P).transpose(2, 1, 0, 3)
                .reshape(KF, P, KD * P).astype(NP_BF16))
        wu_t = (Wu[e].reshape(KD, P, KF, P).transpose(2, 1, 0, 3)
                .reshape(KF, P, KD * P).astype(NP_BF16))
        wd_t = (Wd[e].reshape(KF, P, KD, P).transpose(2, 1, 0, 3)
                .reshape(KD, P, KF * P).astype(NP_BF16))
        maps.append({
            "xt": xt_e,
            "wg": np.ascontiguousarray(wg_t),
            "wu": np.ascontiguousarray(wu_t),
            "wd": np.ascontiguousarray(wd_t),
        })
    return maps


def kernel(x, Wg, Wu, Wd, topk_weights, topk_indices, num_experts):
    x = np.asarray(x, np.float32)
    Wg = np.asarray(Wg, np.float32)
    Wu = np.asarray(Wu, np.float32)
    Wd = np.asarray(Wd, np.float32)
    T_, D_ = x.shape

    combine, idxs = _dispatch(x, topk_weights, topk_indices, num_experts)
    C = _capacity(idxs)

    nc = _build(C)
    res = bass_utils.run_bass_kernel_spmd(nc, _in_maps(x, Wg, Wu, Wd, idxs, C),
                                          list(range(N_CORES)))

    out = np.zeros((T_, D_), np.float32)
    for e in range(len(idxs)):
        n = len(idxs[e])
        if n:
            ye = res.results[e]["yt"][:, :n].T
            out[idxs[e]] += ye * combine[idxs[e], e][:, None]
    return out
